# revision 1
# baseline (speedup 1.0000x reference)
"""DigitCaps routing kernel for Trainium2 (8 NeuronCores, data parallel).

Math note: in the reference, `routers` starts at zero and is only ever
updated by adding a [B, 1] term broadcast along the feature axis, so each
row of `routers` is constant along features at every iteration.  Softmax of
a constant row is exactly uniform (exp(t - t) = 1 elementwise, sum = D), so
`agreements == 1/D` exactly in float32 at every routing iteration, and
`outputs` is identical across all 3 iterations.  The whole module therefore
reduces to a single pass:

    s[b] = (1/D) * sum_d x[b, d]          (D = 9216)
    v[b] = s * |s| / (1 + s^2)            (squash of a scalar)
    out  = norm over last dim of v.reshape(100, 10, 16)

which is a pure memory-bound row reduction over the 590 MB input.  With
S = sum_d x[b, d] this is equivalently v = copysign(S^2 / (S^2 + D^2), S),
which keeps the whole squash on the vector engine (no transcendentals).

Sharding: pure data parallel over batch — 2000 rows of 9216 floats per
core.  Each core computes its rows' squashed scalars v[2000]; the host
concatenates the 8 shards and does the final (16000-element) group norm.
"""

import numpy as np

import concourse.bass as bass
import concourse.bacc as bacc
import concourse.tile as tile
from concourse import mybir
from concourse.bass_utils import run_bass_kernel_spmd

N_CORES = 8
B = 16000
D = 9216  # 1152 * 8
ROWS = B // N_CORES  # 2000 rows per core
P = 128  # SBUF partitions
OUT_FEATURES = 10
OUT_CAPSULES = 16

CSPLIT = 4  # column chunks per [128, D] row tile
BUFS = 14  # chunk-tile pool slots


def build_nc(rows=ROWS, d=D, csplit=CSPLIT, bufs=BUFS, first_split=None):
    """first_split: if set, csplit must be 1; tile 0 alone is split into
    `first_split` column chunks (fast pipeline fill), remaining tiles are
    whole-row DMAs (best steady-state contiguity)."""
    f32 = mybir.dt.float32
    u32 = mybir.dt.uint32
    tiles = [(i * P, min(P, rows - i * P)) for i in range((rows + P - 1) // P)]
    nt = len(tiles)
    n_full = sum(1 for _, r in tiles if r == P)
    assert d % csplit == 0
    cw = d // csplit
    if first_split:
        assert csplit == 1 and d % first_split == 0

    nc = bacc.Bacc(None)
    x = nc.declare_dram_parameter("x", [rows, d], f32, isOutput=False)
    v = nc.declare_dram_parameter("v", [rows], f32, isOutput=True)

    with tile.TileContext(nc) as tc:
        with (
            tc.tile_pool(name="xpool", bufs=bufs) as xpool,
            tc.tile_pool(name="fpool", bufs=max(2, first_split or 0)) as fpool,
            tc.tile_pool(name="small", bufs=1) as small,
        ):
            # sums2[:, i, c] = partial sum of tile i's chunk c
            sums2 = small.tile([P, nt, max(csplit, first_split or 1)], f32)
            sums = small.tile([P, nt], f32)  # S = full row sums
            for i, (r0, r) in enumerate(tiles):
                if first_split and i == 0:
                    fw = d // first_split
                    for c in range(first_split):
                        xc = fpool.tile([P, fw], f32, tag="xc")
                        nc.sync.dma_start(
                            out=xc[:r, :], in_=x[r0 : r0 + r, c * fw : (c + 1) * fw]
                        )
                        nc.vector.reduce_sum(
                            out=sums2[:r, i, c : c + 1],
                            in_=xc[:r, :],
                            axis=mybir.AxisListType.X,
                        )
                    continue
                for c in range(csplit):
                    xt = xpool.tile([P, cw], f32)
                    nc.sync.dma_start(
                        out=xt[:r, :], in_=x[r0 : r0 + r, c * cw : (c + 1) * cw]
                    )
                    if csplit == 1 and first_split:
                        # whole-row reduce goes straight into sums
                        nc.vector.reduce_sum(
                            out=sums[:r, i : i + 1],
                            in_=xt[:r, :],
                            axis=mybir.AxisListType.X,
                        )
                    else:
                        nc.vector.reduce_sum(
                            out=sums2[:r, i, c : c + 1],
                            in_=xt[:r, :],
                            axis=mybir.AxisListType.X,
                        )

            a = small.tile([P, nt], f32)
            denom = small.tile([P, nt], f32)
            rcp = small.tile([P, nt], f32)
            vv = small.tile([P, nt], f32)

            def combine(rsl, csl):
                # sums[rsl, csl] = sum over chunks of sums2[rsl, csl, :]
                if first_split:
                    pass  # handled by combine_first below
                elif csplit == 1:
                    nc.vector.tensor_copy(sums[rsl, csl], sums2[rsl, csl, 0])
                elif csplit == 2:
                    nc.vector.tensor_add(
                        sums[rsl, csl], sums2[rsl, csl, 0], sums2[rsl, csl, 1]
                    )
                elif csplit == 4:
                    nc.vector.tensor_add(
                        a[rsl, csl], sums2[rsl, csl, 0], sums2[rsl, csl, 1]
                    )
                    nc.vector.tensor_add(
                        denom[rsl, csl], sums2[rsl, csl, 2], sums2[rsl, csl, 3]
                    )
                    nc.vector.tensor_add(sums[rsl, csl], a[rsl, csl], denom[rsl, csl])
                else:
                    raise ValueError(csplit)

            def squash(rsl, csl):
                # vv = copysign(S^2 / (S^2 + D^2), S), all on the vector engine
                nc.vector.tensor_mul(a[rsl, csl], sums[rsl, csl], sums[rsl, csl])
                nc.vector.tensor_scalar_add(denom[rsl, csl], a[rsl, csl], float(d * d))
                nc.vector.reciprocal(rcp[rsl, csl], denom[rsl, csl])
                nc.vector.tensor_mul(vv[rsl, csl], a[rsl, csl], rcp[rsl, csl])
                # sign transfer: vv_u32 = (vv_u32 & 0x7fffffff) | (S_u32 & 0x80000000)
                nc.vector.tensor_scalar(
                    out=a.bitcast(u32)[rsl, csl],
                    in0=sums.bitcast(u32)[rsl, csl],
                    scalar1=0x80000000,
                    scalar2=None,
                    op0=mybir.AluOpType.bitwise_and,
                )
                nc.vector.tensor_tensor(
                    out=vv.bitcast(u32)[rsl, csl],
                    in0=vv.bitcast(u32)[rsl, csl],
                    in1=a.bitcast(u32)[rsl, csl],
                    op=mybir.AluOpType.bitwise_or,
                )

            if first_split:
                # fold tile 0's chunk partials into sums[:, 0]
                rsl, c0 = slice(0, P), slice(0, 1)
                if first_split == 2:
                    nc.vector.tensor_add(
                        sums[rsl, c0], sums2[rsl, 0, 0:1], sums2[rsl, 0, 1:2]
                    )
                elif first_split == 4:
                    nc.vector.tensor_add(
                        a[rsl, c0], sums2[rsl, 0, 0:1], sums2[rsl, 0, 1:2]
                    )
                    nc.vector.tensor_add(
                        denom[rsl, c0], sums2[rsl, 0, 2:3], sums2[rsl, 0, 3:4]
                    )
                    nc.vector.tensor_add(sums[rsl, c0], a[rsl, c0], denom[rsl, c0])
                else:
                    raise ValueError(first_split)

            if n_full:
                rsl, csl = slice(0, P), slice(0, n_full)
                combine(rsl, csl)
                squash(rsl, csl)
                # v[n*128 + p] = vv[p, n]
                nc.sync.dma_start(
                    out=v[0 : n_full * P].rearrange("(n p) -> p n", p=P),
                    in_=vv[:, 0:n_full],
                )
            if n_full < nt:
                r0, r = tiles[-1]
                rsl, csl = slice(0, r), slice(n_full, nt)
                combine(rsl, csl)
                squash(rsl, csl)
                nc.sync.dma_start(
                    out=v[r0 : r0 + r].rearrange("(p one) -> p one", one=1),
                    in_=vv[:r, n_full : n_full + 1],
                )
    return nc


def build_nc_raw(rows=ROWS, d=D, csplit=CSPLIT, bufs=BUFS, pair=False, rings=1):
    # pair=True (whole-row DMAs spanning csplit slots for middle tiles)
    # measured consistently WORSE under 8-core HBM contention: the bigger
    # per-tile reduces consume chunkier and the long DMAs do not stream
    # faster.  Interleaved A/B: mixed 226-248us vs single 198-199us.
    """Raw bacc (no Tile): hand-rolled semaphores, minimal prologue/epilogue.

    Sync safety: a DMA's `then_inc(sem, 16)` is 16 independent +1s (one per
    SDMA engine), so a cumulative wait on a shared semaphore can be
    satisfied by a mix of increments from concurrently in-flight DMAs.
    Each buffer slot therefore gets its own semaphore: slot-reuse gating on
    the reduce counter guarantees at most one in-flight DMA per slot sem,
    making `>= 16*k` waits exact.  Every instruction needs at most one
    wait (the slot's previous DMA completion is implied transitively by
    the reduce that consumed it).
    """
    import contextlib
    f32 = mybir.dt.float32
    u32 = mybir.dt.uint32
    tiles = [(i * P, min(P, rows - i * P)) for i in range((rows + P - 1) // P)]
    nt = len(tiles)
    n_full = sum(1 for _, r in tiles if r == P)
    assert d % csplit == 0
    cw = d // csplit

    # Entry schedule: each entry is ONE dma_start into one cw-wide slot +
    # ONE reduce producing one partial into sums2[:, ti, lane].  Widths
    # are tapered at the pipeline ends: the FIRST tile's first chunk is
    # split small->large so the first reduce starts ~2us earlier, and the
    # LAST tile's final chunk is split large->small so the post-last-DMA
    # reduce is ~4x shorter.  Steady-state chunks stay cw wide.  (`pair`
    # whole-row variant removed: measured worse under contention.)
    # Entries: (r0, r, elem0, welem, ti, lane).
    wa = max(32, cw // 4)
    entries = []
    for i, (r0, r) in enumerate(tiles):
        if i == 0 and csplit >= 2 and nt >= 2 and cw - wa >= 32:
            widths = [wa, cw - wa] + [cw] * (csplit - 1)
        elif i == nt - 1 and csplit >= 2 and nt >= 2 and cw - wa >= 32:
            widths = [cw] * (csplit - 1) + [cw - wa, wa]
        else:
            widths = [cw] * csplit
        e0 = 0
        for lane, w in enumerate(widths):
            entries.append((r0, r, e0, w, i, lane))
            e0 += w
    n_lanes = max(lane + 1 for _, _, _, _, _, lane in entries)

    # Slot cursor rotation: one slot per entry; per-slot completion sems.
    slot_of = []
    sem_count = {}  # slot -> DMAs issued on that slot's sem so far
    wait_val = []  # per entry: slot_sem wait value for its consumer reduce
    prev_consumer = {}  # slot -> entry index whose reduce frees it
    gate = []  # per entry: reduce count that must retire before the DMA
    for e in range(len(entries)):
        s = e % bufs
        slot_of.append(s)
        gate.append(prev_consumer.get(s, -1))
        prev_consumer[s] = e
        sem_count[s] = sem_count.get(s, 0) + 1
        wait_val.append(16 * sem_count[s])

    nc = bacc.Bacc(None)
    x = nc.declare_dram_parameter("x", [rows, d], f32, isOutput=False)
    # v2[p, n] = squashed value of row n*128 + p; host unpermutes.
    v = nc.declare_dram_parameter("v", [P, nt], f32, isOutput=True)

    with (
        contextlib.ExitStack() as ctx,
        nc.sbuf_tensor([P, bufs, cw], f32) as xbuf,
        nc.sbuf_tensor([P, nt, n_lanes], f32) as sums2,
        nc.sbuf_tensor([P, nt], f32) as sums,
        nc.sbuf_tensor([P, nt], f32) as a_t,
        nc.sbuf_tensor([P, nt], f32) as den_t,
        nc.sbuf_tensor([P, nt], f32) as rcp_t,
        nc.sbuf_tensor([P, nt], f32) as vv,
        nc.semaphore("red") as red,
        nc.semaphore("sq") as sq,
        nc.semaphore("outd") as outd,
        nc.Block() as block,
    ):
        slot_sem = [
            ctx.enter_context(nc.semaphore(f"dslot{k}")) for k in range(bufs)
        ]

        def issue_dmas(eng, parity):
            # rings=2: even entries on the SP HWDGE ring (sync), odd on the
            # ACT HWDGE ring (scalar).  Per-slot completion sems make the
            # cross-ring arrival order irrelevant; slot-reuse gating keeps
            # at most one in-flight DMA per slot sem as before.
            for e, (r0, r, e0, we, ti, lane) in enumerate(entries):
                if rings > 1 and e % rings != parity:
                    continue
                if gate[e] >= 0:
                    # slots free when the reduce that consumed their
                    # previous occupants retired; red = 1 (memset) + units
                    eng.wait_ge(red, gate[e] + 2)
                s0 = slot_of[e]
                dma = eng.dma_start(
                    out=xbuf[:r, s0, 0:we],
                    in_=x[r0 : r0 + r, e0 : e0 + we],
                )
                dma.then_inc(slot_sem[s0], 16)

        @block.sync
        def _(sync):
            issue_dmas(sync, 0)
            sync.wait_ge(sq, 1)
            with nc.allow_non_contiguous_dma(reason="8KB result store"):
                sync.dma_start(out=v[:, :], in_=vv[:, 0:nt]).then_inc(outd, 16)
            sync.wait_ge(outd, 16)

        if rings > 1:

            @block.scalar
            def _(scalar):
                issue_dmas(scalar, 1)

        @block.vector
        def _(vector):
            # Zero the tail lanes the 80-row tile never writes, so the
            # combine/squash below can run full-width in one pass (the
            # zero lanes squash to 0 and the host discards them).
            vector.memset(sums2[:, :, :], 0.0).then_inc(red, 1)
            vector.wait_ge(red, 1)  # engines have no same-engine interlock
            for e, (r0, r, e0, we, ti, lane) in enumerate(entries):
                s0 = slot_of[e]
                vector.wait_ge(slot_sem[s0], wait_val[e])
                vector.reduce_sum(
                    out=sums2[:r, ti, lane : lane + 1],
                    in_=xbuf[:r, s0, 0:we],
                    axis=mybir.AxisListType.X,
                ).then_inc(red, 1)

            # TRN2 engines have no same-engine memory-hazard interlock:
            # every dependent DVE op below syncs on the shared `red` sem.
            cnt = [len(entries) + 1]

            def step(emit, final_sem=None):
                vector.wait_ge(red, cnt[0])
                inst = emit()
                if final_sem is not None:
                    inst.then_inc(final_sem, 1)  # single update slot
                else:
                    inst.then_inc(red, 1)
                    cnt[0] += 1
                return inst

            def combine(rsl, csl):
                # one lane-axis reduce folds all csplit partials per tile
                step(
                    lambda: vector.reduce_sum(
                        out=sums[rsl, csl],
                        in_=sums2[rsl, csl, :],
                        axis=mybir.AxisListType.X,
                    )
                )

            def squash(rsl, csl, final=False):
                step(
                    lambda: vector.tensor_mul(
                        a_t[rsl, csl], sums[rsl, csl], sums[rsl, csl]
                    )
                )
                step(
                    lambda: vector.tensor_scalar_add(
                        den_t[rsl, csl], a_t[rsl, csl], float(d * d)
                    )
                )
                step(lambda: vector.reciprocal(rcp_t[rsl, csl], den_t[rsl, csl]))
                step(
                    lambda: vector.tensor_mul(
                        vv[rsl, csl], a_t[rsl, csl], rcp_t[rsl, csl]
                    )
                )
                step(
                    lambda: vector.tensor_scalar(
                        out=a_t.bitcast(u32)[rsl, csl],
                        in0=sums.bitcast(u32)[rsl, csl],
                        scalar1=0x80000000,
                        scalar2=None,
                        op0=mybir.AluOpType.bitwise_and,
                    )
                )
                return step(
                    lambda: vector.tensor_tensor(
                        out=vv.bitcast(u32)[rsl, csl],
                        in0=vv.bitcast(u32)[rsl, csl],
                        in1=a_t.bitcast(u32)[rsl, csl],
                        op=mybir.AluOpType.bitwise_or,
                    ),
                    final_sem=sq if final else None,
                )

            combine(slice(0, P), slice(0, nt))
            squash(slice(0, P), slice(0, nt), final=True)

    return nc


_NC_CACHE = {}
IMPL = "raw"  # "raw" (hand-rolled sems, minimal prologue/epilogue) or "tile"


def _get_nc():
    key = IMPL
    if key not in _NC_CACHE:
        nc = build_nc_raw() if IMPL == "raw" else build_nc()
        nc.finalize()  # runs Bacc legalization (wait splitting, reg alloc)
        _NC_CACHE[key] = nc
    return _NC_CACHE[key]


def _gather_v(res_core):
    v = np.asarray(res_core["v"])
    if v.ndim == 2:  # raw impl: v2[p, n] = row n*128 + p
        return v.T.ravel()[:ROWS]
    return v  # tile impl: already [ROWS]


LAST_RESULTS = None  # BassKernelResults of the most recent run (for profiling)


def kernel(inputs: np.ndarray, *, _trace: bool = False, _trace_kwargs=None) -> np.ndarray:
    global LAST_RESULTS
    x = np.ascontiguousarray(np.asarray(inputs, dtype=np.float32)).reshape(B, D)
    in_maps = [{"x": x[c * ROWS : (c + 1) * ROWS]} for c in range(N_CORES)]
    nc = _get_nc()
    res = run_bass_kernel_spmd(
        nc,
        in_maps,
        core_ids=list(range(N_CORES)),
        trace=_trace,
        **(_trace_kwargs or {}),
    )
    LAST_RESULTS = res
    vfull = np.concatenate([_gather_v(res.results[c]) for c in range(N_CORES)])
    out = np.linalg.norm(
        vfull.reshape(-1, OUT_FEATURES, OUT_CAPSULES), axis=-1
    ).astype(np.float32)
    return out

